# revision 18
# baseline (speedup 1.0000x reference)
"""Trainium2 Bass kernel: 3-layer GNN message passing (atom embedding).

Data-parallel over the B*N=400000 point axis across 8 NeuronCores.
Layout on chip: channels-on-partitions. Per point-tile of T=512 points,
a feature tile F[119, T] holds:
  rows 0-95   atomtypes  (k*6+c, neighbor-major)
  rows 96-111 dist       (k)
  rows 112-117 point embedding (updated per layer)
  row  118    constant 1.0 (folds the b1 bias into matmul1)
Matmul1 uses a block-diagonal packed lhsT [119, 104] per neighbor-half
(8 neighbors x 13 output channels); the emb/bias rows contribute to every
neighbor block, so no broadcast op is ever needed. LeakyReLU runs on
ScalarE (Prelu, alpha) for half 0 and VectorE (scalar_tensor_tensor
(z*0.2) max z) for half 1 so the two engines split the dominant cost.
Matmul2 stacks W2 per neighbor (104->6) and folds the neighbor-sum into
the PSUM contraction+accumulation. GroupNorm stats come from a tiny
block-diagonal averaging matmul (G1), and scale/shift+LeakyReLU fuse
into one ScalarE Prelu(scale=gw, bias=gb) op.
"""
import sys

sys.path.insert(0, "/opt/trn_rl_repo")

import numpy as np

D = 6
K = 16
N_LAYERS = 3
C_IN = 13
EPS = 1e-5
SLOPE = 0.2

N_CORES = 8
T = 512            # points per tile (PSUM bank = 512 fp32)
PC = 50000         # points per core
NT = 98            # tiles per core
PP = NT * T        # padded points per core = 50176
GB = 14            # point-tiles per groupnorm batch (84 partitions)
SROWS = 6 * GB     # 84

F_ROWS = 119       # 96 atom + 16 dist + 6 emb + 1 ones


def _pack_weights(W1, b1, W2, b2, gw, gb):
    """Build the packed lhsT / const tensors (host side, a few KB)."""
    lhsT1 = np.zeros((N_LAYERS, 2, F_ROWS, 104), np.float32)
    for i in range(N_LAYERS):
        for half in range(2):
            L = lhsT1[i, half]
            for k8 in range(8):
                k = half * 8 + k8
                cols = slice(k8 * 13, k8 * 13 + 13)
                L[k * 6:(k + 1) * 6, cols] = W1[i, 6:12, :]   # atom rows
                L[96:102, cols] = W1[i, 0:6, :]               # emb rows
                L[102 + k, cols] = W1[i, 12, :]               # dist row
                L[118, cols] = b1[i]                          # bias row
    lhsT1_flat = np.concatenate(
        [lhsT1[i, h] for i in range(N_LAYERS) for h in range(2)], axis=1
    )  # [119, 624]

    lhsT2 = np.zeros((104, N_LAYERS * 6), np.float32)
    for i in range(N_LAYERS):
        for k8 in range(8):
            lhsT2[k8 * 13:k8 * 13 + 13, i * 6:(i + 1) * 6] = W2[i]

    G1 = np.kron(np.eye(2 * GB, dtype=np.float32),
                 np.ones((3, 3), np.float32) / 3.0)  # [6*GB, 6*GB]

    cst = np.zeros((6, 9), np.float32)
    for i in range(N_LAYERS):
        cst[:, i] = 16.0 * b2[i]
        cst[:, 3 + i] = gw[i]
        cst[:, 6 + i] = gb[i]
    cst = np.tile(cst, (GB, 1))  # [6*GB, 9]
    return lhsT1_flat, lhsT2, G1, cst


def _build_nc():
    import concourse.bass as bass
    import concourse.bacc as bacc
    import concourse.mybir as mybir
    from concourse import tile

    bass_ds = bass.ds
    F32 = mybir.dt.float32
    AF = mybir.ActivationFunctionType
    OP = mybir.AluOpType

    import time as _time
    print(f"[kernel] build start {_time.time():.1f}", flush=True)
    nc = bacc.Bacc("TRN2", target_bir_lowering=False)
    atom_e = nc.declare_dram_parameter("atom", [96, PP], F32, isOutput=False)
    dist_e = nc.declare_dram_parameter("dist", [16, PP], F32, isOutput=False)
    l1_e = nc.declare_dram_parameter("lhsT1", [F_ROWS, 624], F32, isOutput=False)
    l2_e = nc.declare_dram_parameter("lhsT2", [104, 18], F32, isOutput=False)
    g1_e = nc.declare_dram_parameter("g1", [SROWS, SROWS], F32, isOutput=False)
    cst_e = nc.declare_dram_parameter("cst", [SROWS, 9], F32, isOutput=False)
    out_e = nc.declare_dram_parameter("out", [6, PP], F32, isOutput=True)

    with tile.TileContext(nc) as tc:
        with tc.tile_pool(name="w", bufs=1) as wp, \
             tc.tile_pool(name="f", bufs=2) as fp, \
             tc.tile_pool(name="h", bufs=4) as hp, \
             tc.tile_pool(name="g", bufs=2) as gp, \
             tc.tile_pool(name="z", bufs=3, space="PSUM") as zp, \
             tc.tile_pool(name="m", bufs=2, space="PSUM") as mp, \
             tc.tile_pool(name="s", bufs=3, space="PSUM") as sp:
            l1 = wp.tile([F_ROWS, 624], F32)
            l2 = wp.tile([104, 18], F32)
            g1 = wp.tile([SROWS, SROWS], F32)
            cst = wp.tile([SROWS, 9], F32)
            eps = wp.tile([128, 1], F32)
            nc.sync.dma_start(out=l1[:], in_=l1_e[:])
            nc.sync.dma_start(out=l2[:], in_=l2_e[:])
            nc.sync.dma_start(out=g1[:], in_=g1_e[:])
            nc.sync.dma_start(out=cst[:], in_=cst_e[:])
            nc.gpsimd.memset(eps[:], EPS)

            with tc.For_i(0, PP, GB * T) as s:
                Fs = []
                for j in range(GB):
                    F = fp.tile([F_ROWS, T], F32, tag=f"F{j}")
                    nc.sync.dma_start(out=F[0:96, :],
                                      in_=atom_e[:, bass_ds(s + j * T, T)])
                    # memset must start 32-aligned: fill [96:119) with 1.0
                    # (emb init + ones row), then dist DMA overwrites 102-117.
                    nc.vector.memset(F[96:F_ROWS, :], 1.0)
                    nc.sync.dma_start(out=F[102:118, :],
                                      in_=dist_e[:, bass_ds(s + j * T, T)])
                    Fs.append(F)
                # batched emb state [84, T]: rows 6j = tile j's embedding
                E = gp.tile([SROWS, T], F32, tag="E")
                nc.vector.memset(E[:], 1.0)

                for i in range(N_LAYERS):
                    stage = gp.tile([SROWS, T], F32, tag="stage")
                    l2s = l2[:, i * 6:(i + 1) * 6]
                    off = (i * 2) * 104
                    for j in range(GB):
                        if i > 0:
                            # refresh this tile's emb rows from E (DMA: no
                            # partition-alignment constraint)
                            nc.sync.dma_start(out=Fs[j][96:102, :],
                                              in_=E[6 * j:6 * j + 6, :])
                        Z0 = zp.tile([128, T], F32, tag="Z")
                        Z1 = zp.tile([128, T], F32, tag="Z")
                        nc.tensor.matmul(Z0[0:104, :], l1[:, off:off + 104],
                                         Fs[j][:], start=True, stop=True)
                        nc.tensor.matmul(Z1[0:104, :],
                                         l1[:, off + 104:off + 208],
                                         Fs[j][:], start=True, stop=True)
                        H0 = hp.tile([104, T], F32, tag="H")
                        H1 = hp.tile([104, T], F32, tag="H")
                        nc.scalar.activation(H0[:], Z0[0:104, :], AF.Prelu,
                                             bias=0.0, scale=1.0, alpha=SLOPE)
                        nc.scalar.activation(H1[:], Z1[0:104, :], AF.Prelu,
                                             bias=0.0, scale=1.0, alpha=SLOPE)
                        msg = mp.tile([6, T], F32, tag="msg")
                        nc.tensor.matmul(msg[0:6, :], l2s, H0[:],
                                         start=True, stop=False)
                        nc.tensor.matmul(msg[0:6, :], l2s, H1[:],
                                         start=False, stop=True)
                        # evac + 16*b2 bias at base 0, then DMA into stage row
                        et = hp.tile([6, T], F32, tag="etmp")
                        nc.vector.tensor_scalar(et[:], msg[0:6, :],
                                                cst[0:6, i:i + 1], None, OP.add)
                        nc.sync.dma_start(out=stage[6 * j:6 * j + 6, :],
                                          in_=et[:])

                    # ---- batched GroupNorm over [84, T] ----
                    sq = gp.tile([SROWS, T], F32, tag="sq")
                    nc.scalar.activation(sq[:], stage[:], AF.Square)
                    mu = sp.tile([SROWS, T], F32, tag="mu")
                    m2 = sp.tile([SROWS, T], F32, tag="mu")
                    nc.tensor.matmul(mu[0:SROWS, :], g1[:], stage[:],
                                     start=True, stop=True)
                    nc.tensor.matmul(m2[0:SROWS, :], g1[:], sq[:],
                                     start=True, stop=True)
                    mus2 = gp.tile([SROWS, T], F32, tag="nmu2")
                    nc.scalar.activation(mus2[:], mu[0:SROWS, :], AF.Square)
                    vpe = gp.tile([SROWS, T], F32, tag="vpe")
                    nc.vector.scalar_tensor_tensor(
                        vpe[:], mus2[:], -1.0, m2[0:SROWS, :],
                        OP.mult, OP.add)
                    sd = gp.tile([SROWS, T], F32, tag="sd")
                    nc.scalar.activation(sd[:], vpe[:], AF.Sqrt,
                                         bias=eps[0:SROWS, 0:1], scale=1.0)
                    rstd = gp.tile([SROWS, T], F32, tag="rstd")
                    nc.vector.reciprocal(rstd[:], sd[:])
                    y = gp.tile([SROWS, T], F32, tag="y")
                    nc.vector.scalar_tensor_tensor(
                        y[:], mu[0:SROWS, :], -1.0, stage[:], OP.mult, OP.add)
                    yr = gp.tile([SROWS, T], F32, tag="yr")
                    nc.vector.tensor_mul(yr[:], y[:], rstd[:])
                    upd = gp.tile([SROWS, T], F32, tag="upd")
                    nc.scalar.activation(upd[:], yr[:], AF.Prelu,
                                         bias=cst[:, 6 + i:7 + i],
                                         scale=cst[:, 3 + i:4 + i], alpha=SLOPE)
                    nc.vector.tensor_add(E[:], E[:], upd[:])

                for j in range(GB):
                    nc.sync.dma_start(out=out_e[:, bass_ds(s + j * T, T)],
                                      in_=E[6 * j:6 * j + 6, :])

    print(f"[kernel] trace done {_time.time():.1f}", flush=True)
    nc.compile()
    print(f"[kernel] bacc compile done {_time.time():.1f}", flush=True)
    return nc


_RUNNER_CACHE = None


def get_runner():
    """Build (once) the jitted 8-core executable. Returns a dict with:
    fn(concat_inputs...) -> out jax arrays, in_names order, zero_outs."""
    global _RUNNER_CACHE
    if _RUNNER_CACHE is not None:
        return _RUNNER_CACHE
    import jax
    import numpy as _np
    from jax.sharding import Mesh, PartitionSpec
    from jax.experimental.shard_map import shard_map
    import concourse.mybir as mybir
    from concourse.bass2jax import (
        install_neuronx_cc_hook, _bass_exec_p, partition_id_tensor)

    nc = _build_nc()
    install_neuronx_cc_hook()
    partition_name = nc.partition_id_tensor.name if nc.partition_id_tensor else None
    in_names, out_names, out_avals, zero_outs = [], [], [], []
    for alloc in nc.m.functions[0].allocations:
        if not isinstance(alloc, mybir.MemoryLocationSet):
            continue
        name = alloc.memorylocations[0].name
        if alloc.kind == "ExternalInput":
            if name != partition_name:
                in_names.append(name)
        elif alloc.kind == "ExternalOutput":
            out_names.append(name)
            shape = tuple(alloc.tensor_shape)
            dtype = mybir.dt.np(alloc.dtype)
            out_avals.append(jax.core.ShapedArray(shape, dtype))
            zero_outs.append(_np.zeros(shape, dtype))
    n_params = len(in_names)
    all_in_names = in_names + out_names
    if partition_name is not None:
        all_in_names.append(partition_name)

    def _body(*args):
        operands = list(args)
        if partition_name is not None:
            operands.append(partition_id_tensor())
        return tuple(_bass_exec_p.bind(
            *operands,
            out_avals=tuple(out_avals),
            in_names=tuple(all_in_names),
            out_names=tuple(out_names),
            lowering_input_output_aliases=(),
            sim_require_finite=True,
            sim_require_nnan=True,
            nc=nc,
        ))

    devices = jax.devices()[:N_CORES]
    mesh = Mesh(_np.asarray(devices), ("core",))
    nin = n_params + len(out_names)
    sharded = jax.jit(shard_map(
        _body, mesh=mesh,
        in_specs=(PartitionSpec("core"),) * nin,
        out_specs=(PartitionSpec("core"),) * len(out_names),
        check_rep=False))
    _RUNNER_CACHE = {
        "fn": sharded, "in_names": in_names, "out_names": out_names,
        "zero_outs": zero_outs, "mesh": mesh,
    }
    return _RUNNER_CACHE


def run_cores(in_maps):
    """Run the 8-core kernel on a list of per-core input dicts."""
    import numpy as _np
    r = get_runner()
    concat_in = [
        _np.concatenate([in_maps[c][name] for c in range(N_CORES)], axis=0)
        for name in r["in_names"]
    ] + [_np.concatenate([z] * N_CORES, axis=0) for z in r["zero_outs"]]
    outs = r["fn"](*concat_in)
    res = []
    for c in range(N_CORES):
        d = {}
        for i, name in enumerate(r["out_names"]):
            full = _np.asarray(outs[i])
            rows = full.shape[0] // N_CORES
            d[name] = full[c * rows:(c + 1) * rows]
        res.append(d)
    return res


def make_in_maps(dist, atomtypes, W1, b1, W2, b2, gw, gb):
    dist = np.asarray(dist, np.float32)
    atomtypes = np.asarray(atomtypes, np.float32)
    B, N, _, _ = atomtypes.shape
    P = B * N
    assert P == N_CORES * PC

    lhsT1, lhsT2, G1, cst = _pack_weights(
        np.asarray(W1, np.float32), np.asarray(b1, np.float32),
        np.asarray(W2, np.float32), np.asarray(b2, np.float32),
        np.asarray(gw, np.float32), np.asarray(gb, np.float32))

    atom_flat = atomtypes.reshape(P, K * D)      # [400000, 96]
    dist_flat = dist.reshape(P, K)               # [400000, 16]

    in_maps = []
    for c in range(N_CORES):
        sl = slice(c * PC, (c + 1) * PC)
        a = np.zeros((PP, 96), np.float32)
        a[:PC] = atom_flat[sl]
        d = np.zeros((PP, 16), np.float32)
        d[:PC] = dist_flat[sl]
        in_maps.append({
            "atom": np.ascontiguousarray(a.T),
            "dist": np.ascontiguousarray(d.T),
            "lhsT1": lhsT1, "lhsT2": lhsT2, "g1": G1, "cst": cst,
        })
    return in_maps, (B, N)


def kernel(dist, atomtypes, W1, b1, W2, b2, gw, gb):
    in_maps, (B, N) = make_in_maps(dist, atomtypes, W1, b1, W2, b2, gw, gb)
    res = run_cores(in_maps)
    outs = [res[c]["out"][:, :PC].T for c in range(N_CORES)]
    return np.concatenate(outs, axis=0).reshape(B, N, D).astype(np.float32)


if __name__ == "__main__":
    rng = np.random.default_rng(0)
    inputs = {
        "dist": rng.random((4, 100000, 16, 1), dtype=np.float32),
        "atomtypes": rng.random((4, 100000, 16, 6), dtype=np.float32),
        "W1": rng.random((3, 13, 13), dtype=np.float32) - 0.5,
        "b1": rng.random((3, 13), dtype=np.float32) - 0.5,
        "W2": rng.random((3, 13, 6), dtype=np.float32) - 0.5,
        "b2": rng.random((3, 6), dtype=np.float32) - 0.5,
        "gw": np.ones((3, 6), np.float32),
        "gb": np.zeros((3, 6), np.float32),
    }
    out = kernel(**inputs)
    print(out.shape, out.dtype)
